# revision 1
# baseline (speedup 1.0000x reference)
"""CTRNN kernel for Trainium2 (Bass/Tile), data-parallel over 8 NeuronCores.

Reference semantics (TAU=1.0 so alpha=1, u carries nothing):
    drive = x @ I_w.T + v                # [B, H], constant over time
    per step: u = drive + z @ H_w.T ; z = tanh(u) ; y = sigmoid(z @ O_w.T + m)

Layout strategy (per core, B=1024):
  - state kept transposed: zT[k-tile][128, B] (h on partitions, batch on free)
  - H_w.T tiles are the stationary matmul operand, zT streams in float32r
    (1 cycle/row at N>=256, ~1.7e-4 rel err/matmul vs 2.5e-3 bf16; the
    recurrence amplifies per-step noise ~10x over 256 steps on weakly-driven
    batch elements, so bf16 is not accurate enough)
  - u accumulated in PSUM fp32; drive added on DVE; tanh on ACT writes f32r z
  - readout logits land in PSUM partition 0 (f32r matmuls cannot write other
    partitions); DVE packs them at 32-aligned strips, sigmoid per 4 steps
  - per 4-step chunk, each [128, 128] b-tile is PE-transposed to [b, (t,o)]
    layout, DVE-compacted (dropping the 20-row strip gaps), and DMA'd out
"""

import os
import sys

for _p in ("/opt/trn_rl_repo", "/root/.axon_site/_ro/trn_rl_repo"):
    if os.path.isdir(_p) and _p not in sys.path:
        sys.path.insert(0, _p)

import ml_dtypes
import numpy as np

N_CORES = 8
B_TOTAL = 8192
B = B_TOTAL // N_CORES  # 1024
H = 512
O = 12
T_STEPS = 256
P = 128
KT = H // P  # 4 k/h' tiles
NB = 512  # moving free-dim per matmul (one PSUM bank of fp32)
BC = B // NB  # 2 batch chunks
CH = 4  # steps per output chunk: step r's readout lands at psum partition 32*r

_BUILT = {}
LAST_RESULTS = None


def _build(t_steps):
    import concourse.mybir as mybir
    import concourse.tile as tile
    from concourse import bacc
    from concourse.masks import make_identity

    f32 = mybir.dt.float32
    f32r = mybir.dt.float32r
    AF = mybir.ActivationFunctionType

    nc = bacc.Bacc(
        "TRN2",
        target_bir_lowering=False,
        debug=False,
        enable_asserts=False,
        num_devices=N_CORES,
    )

    drv_d = nc.dram_tensor("drivet", [H, B], f32, kind="ExternalInput")
    hw_d = nc.dram_tensor("hwt", [H, H], f32r, kind="ExternalInput")
    ow_d = nc.dram_tensor("owt", [H, O], f32r, kind="ExternalInput")
    mb_d = nc.dram_tensor("mb", [P, 1], f32, kind="ExternalInput")
    y_d = nc.dram_tensor("y", [B, t_steps * O], f32, kind="ExternalOutput")

    with tile.TileContext(nc) as tc:
        with (
            tc.tile_pool(name="const", bufs=1) as cpool,
            tc.tile_pool(name="state", bufs=1) as spool,
            tc.tile_pool(name="usb", bufs=4) as upool,
            tc.tile_pool(name="ych", bufs=2) as ypool,
            tc.tile_pool(name="yst", bufs=3) as ystage,
            tc.tile_pool(name="pu", bufs=6, space="PSUM") as pu,
            tc.tile_pool(name="py", bufs=1, space="PSUM") as py,
            tc.tile_pool(name="ptr", bufs=1, space="PSUM") as ptr,
        ):
            # ---- constants ----
            hsb = [
                [
                    cpool.tile([P, P], f32r, name=f"hw_{k}_{h}", tag=f"hw_{k}_{h}")
                    for h in range(KT)
                ]
                for k in range(KT)
            ]
            for k in range(KT):
                for h in range(KT):
                    nc.sync.dma_start(
                        hsb[k][h][:],
                        hw_d[k * P : (k + 1) * P, h * P : (h + 1) * P],
                    )
            osb = [cpool.tile([P, O], f32r, name=f"ow_{k}", tag=f"ow_{k}") for k in range(KT)]
            for k in range(KT):
                nc.sync.dma_start(osb[k][:], ow_d[k * P : (k + 1) * P, :])
            drv = [cpool.tile([P, B], f32, name=f"drv_{h}", tag=f"drv_{h}") for h in range(KT)]
            for h in range(KT):
                nc.sync.dma_start(drv[h][:], drv_d[h * P : (h + 1) * P, :])
            mb = cpool.tile([P, 1], f32, name="mb_sb", tag="mb_sb")
            nc.sync.dma_start(mb[:], mb_d[:, :])
            ident = cpool.tile([P, P], f32, name="ident", tag="ident")
            make_identity(nc, ident[:])

            # ---- state (ping-pong by step parity) ----
            zs = [
                [
                    spool.tile([P, B], f32r, name=f"z{pp}_{k}", tag=f"z{pp}_{k}")
                    for k in range(KT)
                ]
                for pp in range(2)
            ]
            # z0 = 0, via tanh(0) since fp32r consumers need a rounding producer
            zinit = upool.tile([P, B], f32, name="zinit", tag="zinit", bufs=1)
            nc.any.memset(zinit[:], 0.0)
            for k in range(KT):
                nc.scalar.activation(zs[0][k][:], zinit[:], AF.Tanh)

            ychunk = None
            for t in range(t_steps):
                zc = zs[t % 2]
                zn = zs[(t + 1) % 2]
                r = t % CH
                if r == 0:
                    # logits for 4 steps packed at 32-aligned partition strips
                    ychunk = ypool.tile([P, B], f32, name="ychunk", tag="ychunk")

                # ---- two independent batch streams, interleaved so one
                # stream's MM burst hides the other's add+tanh tail ----
                for s in range(BC):
                    sl = slice(s * NB, (s + 1) * NB)
                    for h in range(KT):
                        ups = pu.tile([P, NB], f32, name="ups", tag="ups")
                        for k in range(KT):
                            nc.tensor.matmul(
                                ups[:, :],
                                lhsT=hsb[k][h][:],
                                rhs=zc[k][:, sl],
                                start=(k == 0),
                                stop=(k == KT - 1),
                            )
                        ut = upool.tile([P, NB], f32, name="ut", tag="ut")
                        nc.vector.tensor_add(ut[:], ups[:], drv[h][:, sl])
                        nc.scalar.activation(zn[h][:, sl], ut[:], AF.Tanh)
                    yps = py.tile([O, NB], f32, name="yps", tag="yps")
                    for k in range(KT):
                        nc.tensor.matmul(
                            yps[:, :],
                            lhsT=osb[k][:],
                            rhs=zn[k][:, sl],
                            start=(k == 0),
                            stop=(k == KT - 1),
                        )
                    nc.vector.tensor_copy(ychunk[32 * r : 32 * r + O, sl], yps[:])

                # ---- chunk flush: sigmoid, transpose to [b, (t, o)], DMA out ----
                if r == CH - 1:
                    t0 = t - CH + 1
                    nc.scalar.activation(ychunk[:], ychunk[:], AF.Sigmoid, bias=mb[:])
                    for bt in range(B // P):
                        trp = ptr.tile([P, P], f32, name="trp", tag="trp")
                        nc.tensor.transpose(
                            trp[:],
                            ychunk[:, bt * P : (bt + 1) * P],
                            ident[:],
                        )
                        yout = ystage.tile([P, CH * O], f32, name="yout", tag="yout")
                        nc.vector.tensor_copy(
                            yout[:],
                            trp.rearrange("p (r g) -> p r g", g=32)[:, :, 0:O],
                        )
                        nc.sync.dma_start(
                            y_d[bt * P : (bt + 1) * P, t0 * O : (t0 + CH) * O],
                            yout[:],
                        )
    nc.compile()
    return nc


def _get_nc(t_steps=T_STEPS):
    if t_steps not in _BUILT:
        _BUILT[t_steps] = _build(t_steps)
    return _BUILT[t_steps]


def _prep_in_maps(x, I_w, H_w, O_w, v, m):
    x = np.asarray(x, np.float32)
    I_w = np.asarray(I_w, np.float32)
    H_w = np.asarray(H_w, np.float32)
    O_w = np.asarray(O_w, np.float32)
    v = np.asarray(v, np.float32)
    m = np.asarray(m, np.float32)

    hwt = np.ascontiguousarray(H_w.T.astype(np.float32))
    owt = np.ascontiguousarray(O_w.T.astype(np.float32))
    # m bias replicated at the 32-aligned col-strip offsets used by the
    # packed readout (step r of each 4-step chunk sits at partitions 32r).
    mbcol = np.zeros((P, 1), np.float32)
    for r in range(CH):
        mbcol[32 * r : 32 * r + O, 0] = m

    in_maps = []
    for c in range(N_CORES):
        xc = x[c * B : (c + 1) * B]  # [B, 1]
        drive = xc @ I_w.T + v  # [B, H] fp32
        drivet = np.ascontiguousarray(drive.T.astype(np.float32))  # [H, B]
        in_maps.append({"drivet": drivet, "hwt": hwt, "owt": owt, "mb": mbcol})
    return in_maps


def kernel(x, T, I_w, H_w, O_w, v, m, _t_steps=None, _trace=False):
    global LAST_RESULTS
    from concourse.bass_utils import run_bass_kernel_spmd

    t_steps = int(_t_steps if _t_steps is not None else T)
    nc = _get_nc(t_steps)

    if _trace:
        # NTFF tracing under axon needs the antenv.axon_hooks profile hook;
        # fall back to untraced execution when it's not available.
        try:
            from antenv.axon_hooks import get_axon_ntff_profile_hook

            _trace = get_axon_ntff_profile_hook() is not None
        except Exception:
            _trace = False

    in_maps = _prep_in_maps(x, I_w, H_w, O_w, v, m)
    res = run_bass_kernel_spmd(
        nc, in_maps, core_ids=list(range(N_CORES)), trace=_trace
    )
    LAST_RESULTS = res
    out = np.concatenate(
        [r["y"].reshape(B, t_steps, O) for r in res.results], axis=0
    )
    return out


def bench(x, T, I_w, H_w, O_w, v, m, _t_steps=None, n_iters=5, repeats=1):
    """Time device execution with device-resident inputs (ns, min over iters).

    Replicates bass2jax.run_bass_via_pjrt's shard_map plumbing so the
    repeated timed calls exclude host<->device transfer of inputs/outputs.
    With repeats=R the NEFF is executed R times per dispatch, serialized by
    threading the output buffer through each call — the (R2-R1) slope then
    isolates pure device execution from the axon dispatch floor.
    """
    import jax
    from jax.sharding import Mesh, NamedSharding, PartitionSpec
    from jax.experimental.shard_map import shard_map

    import concourse.mybir as mybir
    from concourse.bass2jax import (
        _bass_exec_p,
        install_neuronx_cc_hook,
        partition_id_tensor,
    )

    t_steps = int(_t_steps if _t_steps is not None else T)
    nc = _get_nc(t_steps)
    install_neuronx_cc_hook()
    in_maps = _prep_in_maps(x, I_w, H_w, O_w, v, m)

    partition_name = (
        nc.partition_id_tensor.name if nc.partition_id_tensor else None
    )
    in_names, out_names, out_avals, zero_outs = [], [], [], []
    for alloc in nc.m.functions[0].allocations:
        if not isinstance(alloc, mybir.MemoryLocationSet):
            continue
        name = alloc.memorylocations[0].name
        if alloc.kind == "ExternalInput":
            if name != partition_name:
                in_names.append(name)
        elif alloc.kind == "ExternalOutput":
            shape = tuple(alloc.tensor_shape)
            dtype = mybir.dt.np(alloc.dtype)
            out_names.append(name)
            out_avals.append(jax.core.ShapedArray(shape, dtype))
            zero_outs.append(np.zeros(shape, dtype))
    n_params = len(in_names)
    in_names = in_names + out_names
    if partition_name is not None:
        in_names.append(partition_name)

    def _body(*args):
        ins = list(args[:n_params])
        outs = list(args[n_params:])
        for _ in range(repeats):
            operands = ins + outs
            if partition_name is not None:
                operands.append(partition_id_tensor())
            outs = list(
                _bass_exec_p.bind(
                    *operands,
                    out_avals=tuple(out_avals),
                    in_names=tuple(in_names),
                    out_names=tuple(out_names),
                    lowering_input_output_aliases=(),
                    sim_require_finite=True,
                    sim_require_nnan=True,
                    nc=nc,
                )
            )
        return tuple(outs)

    devices = jax.devices()[:N_CORES]
    mesh = Mesh(np.asarray(devices), ("core",))
    n_outs = len(out_names)
    donate = tuple(range(n_params, n_params + n_outs))
    sharded = jax.jit(
        shard_map(
            _body,
            mesh=mesh,
            in_specs=(PartitionSpec("core"),) * (n_params + n_outs),
            out_specs=(PartitionSpec("core"),) * n_outs,
            check_rep=False,
        ),
        donate_argnums=donate,
        keep_unused=True,
    )
    sh = NamedSharding(mesh, PartitionSpec("core"))
    concat_in = [
        np.concatenate([np.asarray(in_maps[c][in_names[i]]) for c in range(N_CORES)], axis=0)
        for i in range(n_params)
    ]
    dev_in = [jax.device_put(a, sh) for a in concat_in]
    big_zeros = [np.zeros((N_CORES * z.shape[0], *z.shape[1:]), z.dtype) for z in zero_outs]

    import time as _time

    times = []
    out = None
    for it in range(n_iters + 1):  # first call = compile/warmup, excluded
        dev_zeros = [jax.device_put(z, sh) for z in big_zeros]
        jax.block_until_ready(dev_zeros)
        t0 = _time.perf_counter()
        out = sharded(*dev_in, *dev_zeros)
        jax.block_until_ready(out)
        dt = _time.perf_counter() - t0
        if it > 0:
            times.append(dt)
    result = np.asarray(out[0]).reshape(N_CORES, B, t_steps, O).reshape(B_TOTAL, t_steps, O)
    return int(min(times) * 1e9), times, result




# revision 3
# speedup vs baseline: 116.5262x; 116.5262x over previous
"""CTRNN kernel for Trainium2 (Bass/Tile), grid + interpolation, 8 NeuronCores.

Reference semantics (TAU=1.0 so alpha=1, u carries nothing):
    drive = x @ I_w.T + v                # [B, H], constant over time
    per step: u = drive + z @ H_w.T ; z = tanh(u) ; y = sigmoid(z @ O_w.T + m)

Key structural fact: x is [B, 1], so every batch element's whole trajectory is
a smooth scalar function y[b,t,o] = F(x_b).  Instead of running the recurrence
on all 8192 batch rows, run it on a 2048-point grid over the observed range of
x (256 points per core, quantile-sharded so each core also owns the 1024 batch
elements falling in its grid range) and reconstruct every batch element by
4-point Lagrange interpolation of the grid trajectories (verified 4e-6 abs err
vs the fp32 reference in numpy; the interp matmul runs in bf16, adding ~2e-3).

Per-core layout (NG=256 grid points, BPC=1024 batch rows):
  - state zT[k][128, NG] f32r (h on partitions, grid on free); H_w.T tiles
    stationary, z streams at 1 cycle/row (f32r fast path needs moving>=256,
    which is what pins the grid at 256/core)
  - u accumulates in PSUM; drive added on DVE for h'-tiles 0-2; tile 3's drive
    enters the PSUM accumulation via a K=2 matmul (rows = [I_w, v] x [grid; 1])
    issued a step early, so the critical-path tile needs no DVE hop before tanh
  - readout logits [12, NG] per step packed into 32-part strips of a 4-step
    ychunk; sigmoid(+m strip bias) on ACT, PE-transpose to [g, (t,o)], DVE
    compacts/casts to bf16 Ygrid
  - every 4 chunks (16 steps) the bf16 interp matmul (stationary Winterp tiles
    [g,b], 4 Lagrange weights per column) produces y[b, 192] in PSUM; ACT
    copies to SBUF and it DMAs out
  - PE work/step ~= 4096 (rec) + 1024 (readout) + 256 (drive) + ~320 (amortized
    transpose+interp) cycles @ 2.4 GHz ~= 2.4us; readout/drive/interp are
    emitted after the next step's rec so they fill the tanh-latency gap
"""

import os
import sys

for _p in ("/opt/trn_rl_repo", "/root/.axon_site/_ro/trn_rl_repo"):
    if os.path.isdir(_p) and _p not in sys.path:
        sys.path.insert(0, _p)

import ml_dtypes
import numpy as np

N_CORES = 8
B_TOTAL = 8192
BPC = B_TOTAL // N_CORES  # 1024 batch rows per core
H = 512
O = 12
T_STEPS = 256
P = 128
KT = H // P  # 4 k/h' tiles
NG = 256  # grid points per core
GCH = 4  # chunks per output group (16 steps -> 192 output cols)

_BUILT = {}
LAST_RESULTS = None


def _build(t_steps):
    import concourse.mybir as mybir
    import concourse.tile as tile
    from concourse import bacc
    from concourse.masks import make_identity

    assert t_steps % 4 == 0
    nch = t_steps // 4  # chunks
    f32 = mybir.dt.float32
    f32r = mybir.dt.float32r
    bf16 = mybir.dt.bfloat16
    AF = mybir.ActivationFunctionType

    nc = bacc.Bacc(
        "TRN2",
        target_bir_lowering=False,
        debug=False,
        enable_asserts=False,
        num_devices=N_CORES,
    )

    drv_d = nc.dram_tensor("drvt", [P, KT * NG], f32, kind="ExternalInput")
    hw_d = nc.dram_tensor("hwt", [P, KT * KT * P], f32r, kind="ExternalInput")
    ow_d = nc.dram_tensor("owt", [P, KT * O], f32r, kind="ExternalInput")
    dw3_d = nc.dram_tensor("dw3", [2, P], f32r, kind="ExternalInput")
    xg1_d = nc.dram_tensor("xg1", [2, NG], f32r, kind="ExternalInput")
    mb_d = nc.dram_tensor("mb", [P, 1], f32, kind="ExternalInput")
    wi_d = nc.dram_tensor("wint", [P, 2 * (BPC // P) * P], bf16, kind="ExternalInput")
    y_d = nc.dram_tensor("y", [BPC, t_steps * O], f32, kind="ExternalOutput")

    with tile.TileContext(nc) as tc:
        with (
            tc.tile_pool(name="const", bufs=1) as cpool,
            tc.tile_pool(name="state", bufs=1) as spool,
            tc.tile_pool(name="ych", bufs=2) as ypool,
            tc.tile_pool(name="ygr", bufs=2) as gpool,
            tc.tile_pool(name="stg", bufs=6) as stpool,
            tc.tile_pool(name="pb", bufs=1, space="PSUM") as ppool,
        ):
            # ---- constants (host pre-tiled; one DMA each) ----
            drvsb = cpool.tile([P, KT * NG], f32, name="drvsb", tag="drvsb")
            nc.sync.dma_start(drvsb[:], drv_d[:, :])
            dw3 = cpool.tile([2, P], f32r, name="dw3", tag="dw3")
            nc.sync.dma_start(dw3[:], dw3_d[:, :])
            xg1 = cpool.tile([2, NG], f32r, name="xg1", tag="xg1")
            nc.sync.dma_start(xg1[:], xg1_d[:, :])
            hwsb = cpool.tile([P, KT * KT * P], f32r, name="hwsb", tag="hwsb")
            nc.sync.dma_start(hwsb[:], hw_d[:, :])
            owsb = cpool.tile([P, KT * O], f32r, name="owsb", tag="owsb")
            nc.sync.dma_start(owsb[:], ow_d[:, :])
            mb = cpool.tile([P, 1], f32, name="mb", tag="mb")
            nc.sync.dma_start(mb[:], mb_d[:, :])
            wisb = cpool.tile([P, 2 * (BPC // P) * P], bf16, name="wisb", tag="wisb")
            nc.sync.dma_start(wisb[:], wi_d[:, :])
            ident = cpool.tile([P, P], f32, name="ident", tag="ident")
            make_identity(nc, ident[:])

            def hw_tile(k, h):
                return hwsb[:, (k * KT + h) * P : (k * KT + h + 1) * P]

            def ow_tile(k):
                return owsb[:, k * O : (k + 1) * O]

            def wi_tile(gk, bt):
                j = gk * (BPC // P) + bt
                return wisb[:, j * P : (j + 1) * P]

            def drv_sl(h):
                return drvsb[:, h * NG : (h + 1) * NG]

            # ---- state (ping-pong by step parity) ----
            zs = [
                [
                    spool.tile([P, NG], f32r, name=f"z{pp}_{k}", tag=f"z{pp}_{k}")
                    for k in range(KT)
                ]
                for pp in range(2)
            ]

            # ---- PSUM banks (manual full-bank tiles, sliced) ----
            ub = [ppool.tile([P, 512], f32, name=f"ub{j}", tag=f"ub{j}") for j in range(5)]
            u_sl = [ub[0][:, 0:NG], ub[1][:, 0:NG], ub[2][:, 0:NG]]  # h' 0..2
            u3 = [ub[3][:, 0:NG], ub[4][:, 0:NG]]  # h'=3, by step parity
            ypsb = ppool.tile([P, 512], f32, name="ypsb", tag="ypsb")
            yps = [ypsb[0:O, 0:NG], ypsb[0:O, NG : 2 * NG]]  # by t parity
            trpb = ppool.tile([P, 512], f32, name="trpb", tag="trpb")
            trp = [trpb[:, 0:P], trpb[:, P : 2 * P]]
            ipb = ppool.tile([P, 512], f32, name="ipb", tag="ipb")
            ips = [ipb[:, 0:GCH * 48], ipb[:, 256 : 256 + GCH * 48]]

            # ---- deferred work queue (PE gap fillers) ----
            work = []
            ygr_ref = [None]
            n_bt = BPC // P

            def emit_chunk(c, ychunk):
                # sigmoid over the packed logit strips (+m strip bias), then
                # transpose each 128-wide grid block and compact to bf16 Ygrid
                nc.scalar.activation(ychunk[:], ychunk[:], AF.Sigmoid, bias=mb[:])
                gi = c % GCH
                if gi == 0:
                    ygr_ref[0] = [
                        gpool.tile([P, GCH * 48], bf16, name=f"yg{gk}", tag=f"yg{gk}")
                        for gk in range(2)
                    ]
                yg = ygr_ref[0]
                for gk in range(2):
                    nc.tensor.transpose(
                        trp[gk][:, :], ychunk[:, gk * P : (gk + 1) * P], ident[:]
                    )
                    dst = yg[gk][:, gi * 48 : (gi + 1) * 48].rearrange(
                        "p (r o) -> p r o", o=O
                    )
                    src = trp[gk].rearrange("p (r g) -> p r g", g=32)[:, :, 0:O]
                    nc.vector.tensor_copy(dst, src)

            def emit_btile(g, bt, gw, task_idx):
                yg = ygr_ref[0]
                dst_ps = ips[task_idx % 2][:, 0:gw]
                for gk in range(2):
                    nc.tensor.matmul(
                        dst_ps,
                        lhsT=wi_tile(gk, bt),
                        rhs=yg[gk][:, 0:gw],
                        start=(gk == 0),
                        stop=(gk == 1),
                    )
                stg = stpool.tile([P, GCH * 48], f32, name="stg", tag="stg")
                nc.scalar.activation(stg[:, 0:gw], dst_ps, AF.Copy)
                nc.sync.dma_start(
                    y_d[bt * P : (bt + 1) * P, g * (GCH * 48) : g * (GCH * 48) + gw],
                    stg[:, 0:gw],
                )

            bt_counter = [0]

            def pop_work():
                # <=1 chunk task, then <=2 btile tasks per iteration
                if work and work[0][0] == "chunk":
                    _, c, ych = work.pop(0)
                    emit_chunk(c, ych)
                    if c == nch - 1 or c % GCH == GCH - 1:
                        g = c // GCH
                        gw = 48 * (c % GCH + 1)
                        for bt in range(n_bt):
                            work.append(("bt", g, bt, gw))
                nb = 0
                while work and work[0][0] == "bt" and nb < 2:
                    _, g, bt, gw = work.pop(0)
                    emit_btile(g, bt, gw, bt_counter[0])
                    bt_counter[0] += 1
                    nb += 1

            # ---- step 1: z_1 = tanh(drive) ----
            for k in range(KT):
                nc.scalar.activation(zs[1][k][:], drv_sl(k), AF.Tanh)
            if t_steps >= 2:
                # open u_2's tile-3 accumulation with the drive matmul
                nc.tensor.matmul(
                    u3[0], lhsT=dw3[:], rhs=xg1[:], start=True, stop=False
                )

            ychunk = None
            for i in range(2, t_steps + 2):
                t = i - 1  # readout/pack target this iteration
                zc = zs[(i - 1) % 2]
                zn = zs[i % 2]
                if i <= t_steps:
                    # recurrence: u_i = drive + H_w.T-tiles @ z_{i-1}
                    for h in range(3):
                        for k in range(KT):
                            nc.tensor.matmul(
                                u_sl[h],
                                lhsT=hw_tile(k, h),
                                rhs=zc[k][:, :],
                                start=(k == 0),
                                stop=(k == KT - 1),
                            )
                    for k in range(KT):
                        nc.tensor.matmul(
                            u3[i % 2],
                            lhsT=hw_tile(k, 3),
                            rhs=zc[k][:, :],
                            start=False,
                            stop=(k == KT - 1),
                        )
                # readout of z_{i-1} -> y_t logits (fills the tanh-wait gap)
                for k in range(KT):
                    nc.tensor.matmul(
                        yps[t % 2],
                        lhsT=ow_tile(k),
                        rhs=zc[k][:, :],
                        start=(k == 0),
                        stop=(k == KT - 1),
                    )
                if i < t_steps:
                    nc.tensor.matmul(
                        u3[(i + 1) % 2], lhsT=dw3[:], rhs=xg1[:], start=True, stop=False
                    )
                if i <= t_steps:
                    for h in range(3):
                        nc.vector.tensor_add(u_sl[h], u_sl[h], drv_sl(h))
                    for h in range(3):
                        nc.scalar.activation(zn[h][:], u_sl[h], AF.Tanh)
                    nc.scalar.activation(zn[3][:], u3[i % 2], AF.Tanh)
                # pop deferred work first so a chunk enqueued this iteration is
                # emitted next iteration (its sigmoid dep will be long resolved
                # by the time PE reaches the transposes)
                pop_work()
                # pack y_t logits into the 4-step chunk at 32-part strips
                r = (t - 1) % 4
                if r == 0:
                    ychunk = ypool.tile([P, NG], f32, name="ychunk", tag="ychunk")
                    nc.gpsimd.memset(ychunk[:], 0.0)
                nc.vector.tensor_copy(ychunk[32 * r : 32 * r + O, :], yps[t % 2])
                if r == 3:
                    work.append(("chunk", (t - 1) // 4, ychunk))
            while work:
                pop_work()
    nc.compile()
    return nc


def _get_nc(t_steps=T_STEPS):
    if t_steps not in _BUILT:
        _BUILT[t_steps] = _build(t_steps)
    return _BUILT[t_steps]


def _prep_in_maps(x, I_w, H_w, O_w, v, m):
    x = np.asarray(x, np.float64)
    I_w = np.asarray(I_w, np.float64)
    H_w = np.asarray(H_w, np.float64)
    O_w = np.asarray(O_w, np.float64)
    v = np.asarray(v, np.float64)
    m = np.asarray(m, np.float64)

    hwt = H_w.T  # [H, H]
    hw_tiled = np.concatenate(
        [
            hwt[k * P : (k + 1) * P, h * P : (h + 1) * P]
            for k in range(KT)
            for h in range(KT)
        ],
        axis=1,
    ).astype(np.float32)  # [128, KT*KT*128]
    owt = O_w.T  # [H, O]
    ow_tiled = np.concatenate(
        [owt[k * P : (k + 1) * P, :] for k in range(KT)], axis=1
    ).astype(np.float32)  # [128, KT*O]
    mbcol = np.zeros((P, 1), np.float32)
    for r in range(4):
        mbcol[32 * r : 32 * r + O, 0] = m

    xs = x[:, 0]
    order = np.argsort(xs, kind="stable")

    in_maps = []
    for c in range(N_CORES):
        idx = order[c * BPC : (c + 1) * BPC]
        xc = xs[idx]  # sorted within the core
        lo, hi = xc[0], xc[-1]
        hstep = max(hi - lo, 1e-6) / (NG - 4)
        glo = lo - 1.5 * hstep
        grid = glo + hstep * np.arange(NG)

        drive = I_w[:, 0:1] * grid[None, :] + v[:, None]  # [H, NG]
        drv_tiled = np.concatenate(
            [drive[h * P : (h + 1) * P, :] for h in range(KT)], axis=1
        ).astype(np.float32)
        dw3 = np.stack([I_w[384:512, 0], v[384:512]]).astype(np.float32)  # [2,128]
        xg1 = np.stack([grid, np.ones(NG)]).astype(np.float32)  # [2,NG]

        # 4-pt Lagrange interpolation weights on the uniform grid
        tq = (xc - glo) / hstep
        j = np.clip(np.floor(tq).astype(np.int64), 1, NG - 3)
        tt = tq - j  # in [0,1] for interior points
        wm1 = -tt * (tt - 1.0) * (tt - 2.0) / 6.0
        w0 = (tt + 1.0) * (tt - 1.0) * (tt - 2.0) / 2.0
        w1 = -(tt + 1.0) * tt * (tt - 2.0) / 2.0
        w2 = (tt + 1.0) * tt * (tt - 1.0) / 6.0
        W = np.zeros((NG, BPC), np.float64)
        cols = np.arange(BPC)
        W[j - 1, cols] = wm1
        W[j, cols] = w0
        W[j + 1, cols] = w1
        W[j + 2, cols] = w2
        wi_tiled = np.concatenate(
            [
                W[gk * P : (gk + 1) * P, bt * P : (bt + 1) * P]
                for gk in range(2)
                for bt in range(BPC // P)
            ],
            axis=1,
        ).astype(ml_dtypes.bfloat16)

        in_maps.append(
            {
                "drvt": drv_tiled,
                "hwt": hw_tiled,
                "owt": ow_tiled,
                "dw3": dw3,
                "xg1": xg1,
                "mb": mbcol,
                "wint": wi_tiled,
            }
        )
    return in_maps, order


def kernel(x, T, I_w, H_w, O_w, v, m, _t_steps=None, _trace=False):
    global LAST_RESULTS
    from concourse.bass_utils import run_bass_kernel_spmd

    t_steps = int(_t_steps if _t_steps is not None else T)
    nc = _get_nc(t_steps)

    if _trace:
        try:
            from antenv.axon_hooks import get_axon_ntff_profile_hook

            _trace = get_axon_ntff_profile_hook() is not None
        except Exception:
            _trace = False

    in_maps, order = _prep_in_maps(x, I_w, H_w, O_w, v, m)
    res = run_bass_kernel_spmd(
        nc, in_maps, core_ids=list(range(N_CORES)), trace=_trace
    )
    LAST_RESULTS = res
    out = np.empty((B_TOTAL, t_steps, O), np.float32)
    for c in range(N_CORES):
        out[order[c * BPC : (c + 1) * BPC]] = res.results[c]["y"].reshape(
            BPC, t_steps, O
        )
    return out


def bench(x, T, I_w, H_w, O_w, v, m, _t_steps=None, n_iters=5, repeats=1):
    """Time device execution with device-resident inputs (ns, min over iters).

    Replicates bass2jax.run_bass_via_pjrt's shard_map plumbing so the
    repeated timed calls exclude host<->device transfer of inputs/outputs.
    """
    import jax
    from jax.sharding import Mesh, NamedSharding, PartitionSpec
    from jax.experimental.shard_map import shard_map

    import concourse.mybir as mybir
    from concourse.bass2jax import (
        _bass_exec_p,
        install_neuronx_cc_hook,
        partition_id_tensor,
    )

    t_steps = int(_t_steps if _t_steps is not None else T)
    nc = _get_nc(t_steps)
    install_neuronx_cc_hook()
    in_maps, order = _prep_in_maps(x, I_w, H_w, O_w, v, m)

    partition_name = (
        nc.partition_id_tensor.name if nc.partition_id_tensor else None
    )
    in_names, out_names, out_avals, zero_outs = [], [], [], []
    for alloc in nc.m.functions[0].allocations:
        if not isinstance(alloc, mybir.MemoryLocationSet):
            continue
        name = alloc.memorylocations[0].name
        if alloc.kind == "ExternalInput":
            if name != partition_name:
                in_names.append(name)
        elif alloc.kind == "ExternalOutput":
            shape = tuple(alloc.tensor_shape)
            dtype = mybir.dt.np(alloc.dtype)
            out_names.append(name)
            out_avals.append(jax.core.ShapedArray(shape, dtype))
            zero_outs.append(np.zeros(shape, dtype))
    n_params = len(in_names)
    in_names = in_names + out_names
    if partition_name is not None:
        in_names.append(partition_name)

    def _body(*args):
        ins = list(args[:n_params])
        outs = list(args[n_params:])
        for _ in range(repeats):
            operands = ins + outs
            if partition_name is not None:
                operands.append(partition_id_tensor())
            outs = list(
                _bass_exec_p.bind(
                    *operands,
                    out_avals=tuple(out_avals),
                    in_names=tuple(in_names),
                    out_names=tuple(out_names),
                    lowering_input_output_aliases=(),
                    sim_require_finite=True,
                    sim_require_nnan=True,
                    nc=nc,
                )
            )
        return tuple(outs)

    devices = jax.devices()[:N_CORES]
    mesh = Mesh(np.asarray(devices), ("core",))
    n_outs = len(out_names)
    donate = tuple(range(n_params, n_params + n_outs))
    sharded = jax.jit(
        shard_map(
            _body,
            mesh=mesh,
            in_specs=(PartitionSpec("core"),) * (n_params + n_outs),
            out_specs=(PartitionSpec("core"),) * n_outs,
            check_rep=False,
        ),
        donate_argnums=donate,
        keep_unused=True,
    )
    sh = NamedSharding(mesh, PartitionSpec("core"))
    concat_in = [
        np.concatenate([np.asarray(in_maps[c][in_names[i]]) for c in range(N_CORES)], axis=0)
        for i in range(n_params)
    ]
    dev_in = [jax.device_put(a, sh) for a in concat_in]
    big_zeros = [np.zeros((N_CORES * z.shape[0], *z.shape[1:]), z.dtype) for z in zero_outs]

    import time as _time

    times = []
    out = None
    for it in range(n_iters + 1):  # first call = compile/warmup, excluded
        dev_zeros = [jax.device_put(z, sh) for z in big_zeros]
        jax.block_until_ready(dev_zeros)
        t0 = _time.perf_counter()
        out = sharded(*dev_in, *dev_zeros)
        jax.block_until_ready(out)
        dt = _time.perf_counter() - t0
        if it > 0:
            times.append(dt)
    ydev = np.asarray(out[0]).reshape(N_CORES, BPC, t_steps, O)
    result = np.empty((B_TOTAL, t_steps, O), np.float32)
    for c in range(N_CORES):
        result[order[c * BPC : (c + 1) * BPC]] = ydev[c]
    return int(min(times) * 1e9), times, result
